# revision 8
# baseline (speedup 1.0000x reference)
"""Trainium2 Bass kernel for nn_MPNNLayer (gnn_message_passing).

Strategy (8 NeuronCores, SPMD, no collectives):
  - Host packs the 20000 nodes into 160 "windows" of <=128 nodes each,
    balanced so every window has roughly equal incident-edge count.
    20 windows per core -> each core owns a disjoint 2560-node slice
    (in permuted order).
  - Edges are grouped by the window of their source node, padded to a
    fixed per-window quota (T_win tiles of 128 edges).  Every core then
    runs an IDENTICAL static schedule.
  - Device: edge MLP runs weight-stationary with features on partitions
    (h_E is passed pre-transposed), the scatter/segment-sum is a one-hot
    matmul on the TensorEngine accumulating in PSUM per window, then the
    node stage (LN -> MLP -> LN) runs per window.  Output rows are
    inverse-permuted on the host.
"""

import sys
import heapq

import numpy as np

for _p in ("/opt/trn_rl_repo",):
    if _p not in sys.path:
        sys.path.insert(0, _p)

N_NODES, N_EDGES, H, IN = 20000, 320000, 128, 256
SCALE, EPS = 30.0, 1e-5
NCORES = 8
W_PER_CORE = 20            # node windows per core (128 node slots each)
NW = NCORES * W_PER_CORE   # 160 windows globally


# ---------------------------------------------------------------- host prep

def _pack_nodes(deg):
    """Assign each node to one of NW windows (<=128 nodes each), greedily
    balancing total edge load.  Returns win_of, slot_of, max_load."""
    order = np.argsort(-deg, kind="stable")
    win_of = np.empty(N_NODES, np.int32)
    slot_of = np.empty(N_NODES, np.int32)
    counts = np.zeros(NW, np.int32)
    heap = [(0, w) for w in range(NW)]
    heapq.heapify(heap)
    for n in order:
        while True:
            load, w = heapq.heappop(heap)
            if counts[w] < 128:
                break
        win_of[n] = w
        slot_of[n] = counts[w]
        counts[w] += 1
        heapq.heappush(heap, (load + int(deg[n]), w))
    loads = np.zeros(NW, np.int64)
    np.add.at(loads, win_of, deg)
    return win_of, slot_of, int(loads.max())


def prep(h_V, h_E, edge_idx):
    """All index gymnastics + data staging.  Returns per-core device arrays
    plus the node permutation needed to unshuffle the output."""
    h_V = np.asarray(h_V, np.float32)
    h_E = np.asarray(h_E, np.float32)
    src = np.asarray(edge_idx[0]).astype(np.int64)
    deg = np.bincount(src, minlength=N_NODES).astype(np.int64)

    win_of, slot_of, max_load = _pack_nodes(deg)
    T_win = max(16, 4 * int(np.ceil(max_load / 512.0)))  # edge tiles / window
    Q = T_win * 128                                      # edge quota / window

    # group edges by window, pad to quota
    wedge = win_of[src]
    order_e = np.argsort(wedge, kind="stable")
    wcounts = np.bincount(wedge, minlength=NW)
    starts = np.zeros(NW + 1, np.int64)
    starts[1:] = np.cumsum(wcounts)
    eidx = np.full((NW, Q), -1, np.int64)
    for w in range(NW):
        eidx[w, : wcounts[w]] = order_e[starts[w] : starts[w + 1]]
    valid = eidx >= 0

    # local (in-window) source slot per scheduled edge; sentinel for pads
    lsrc = np.full((NW, Q), 1.0e6, np.float32)
    lsrc[valid] = slot_of[src[eidx[valid]]].astype(np.float32)

    # gather + transpose h_E into per-core blocks of (256 x 512)
    hEg = np.zeros((NW, Q, IN), np.float32)
    hEg[valid] = h_E[eidx[valid]]
    BLK = W_PER_CORE * Q // 512
    hEb = np.ascontiguousarray(
        hEg.reshape(NCORES, BLK, 512, IN).transpose(0, 1, 3, 2)
    )

    # srcf[c, w, p, t] = local src of edge (w*Q + t*128 + p)
    srcf = np.ascontiguousarray(
        lsrc.reshape(NCORES, W_PER_CORE, T_win, 128).transpose(0, 1, 3, 2)
    )

    # node permutation: perm[w, slot] = original node id (-1 = dummy)
    perm = np.full((NW, 128), -1, np.int64)
    perm[win_of, slot_of] = np.arange(N_NODES)
    pm = perm >= 0

    hVp = np.zeros((NW, 128, H), np.float32)
    hVp[pm] = h_V[perm[pm]]
    hVp = np.ascontiguousarray(hVp.reshape(NCORES, W_PER_CORE * 128, H))

    degf = np.zeros((NW, 128, 1), np.float32)
    degf[pm, 0] = deg[perm[pm]].astype(np.float32)
    degf = np.ascontiguousarray(degf.reshape(NCORES, W_PER_CORE, 128, 1))

    return dict(T_win=T_win, hEb=hEb, srcf=srcf, hVp=hVp, degf=degf,
                perm=perm, pm=pm)


def _weight_arrays(W1_w, W1_b, W2_w, W2_b, W3_w, W3_b,
                   n1_g, n1_b, d1_w, d1_b, d2_w, d2_b, n2_g, n2_b):
    f = np.float32
    t = lambda v: np.ascontiguousarray(np.tile(np.asarray(v, f), (128, 1)))
    return {
        "W1s": np.ascontiguousarray(
            np.asarray(W1_w, f).reshape(2, 128, H).transpose(1, 0, 2)),
        "W2s": np.ascontiguousarray(np.asarray(W2_w, f)),
        "W3s": np.ascontiguousarray(np.asarray(W3_w, f) / SCALE),
        "d1s": np.ascontiguousarray(np.asarray(d1_w, f)),
        "d2s": np.ascontiguousarray(
            np.asarray(d2_w, f).reshape(4, 128, H).transpose(1, 0, 2)),
        "b1c": np.asarray(W1_b, f).reshape(128, 1).copy(),
        "b2c": np.asarray(W2_b, f).reshape(128, 1).copy(),
        "d1bc": np.ascontiguousarray(np.asarray(d1_b, f).reshape(4, 128).T),
        "B3s": t(np.asarray(W3_b, f) / SCALE),
        "B2d": t(d2_b),
        "G1": t(n1_g), "B1n": t(n1_b), "G2": t(n2_g), "B2n": t(n2_b),
        "IOTA": np.ascontiguousarray(
            np.tile(np.arange(128, dtype=f), (128, 1))),
        "IDN": np.eye(128, dtype=f),
        "EPSc": np.full((128, 1), EPS, f),
    }


# ------------------------------------------------------------- bass program

_BUILD_CACHE = {}


def build_nc(T_win, repeat=1):
    if (T_win, repeat) in _BUILD_CACHE:
        return _BUILD_CACHE[(T_win, repeat)]

    from contextlib import ExitStack
    import concourse.bass as bass
    import concourse.tile as tile
    from concourse import bacc, mybir

    f32 = mybir.dt.float32
    AF = mybir.ActivationFunctionType
    OP = mybir.AluOpType
    AX = mybir.AxisListType.X
    PSUM = bass.MemorySpace.PSUM

    SPB = T_win // 4                    # 512-edge blocks per window
    BLK = W_PER_CORE * SPB              # blocks per core

    nc = bacc.Bacc("TRN2", target_bir_lowering=False, debug=False)

    hE_d = nc.dram_tensor("hE", [BLK, IN, 512], f32, kind="ExternalInput").ap()
    src_d = nc.dram_tensor("srcf", [W_PER_CORE, 128, T_win], f32,
                           kind="ExternalInput").ap()
    hV_d = nc.dram_tensor("hV", [W_PER_CORE * 128, H], f32,
                          kind="ExternalInput").ap()
    deg_d = nc.dram_tensor("deg", [W_PER_CORE, 128, 1], f32,
                           kind="ExternalInput").ap()
    wd = {}
    for name, shape in [
        ("W1s", [128, 2, 128]), ("W2s", [128, 128]), ("W3s", [128, 128]),
        ("d1s", [128, 512]), ("d2s", [128, 4, 128]),
        ("b1c", [128, 1]), ("b2c", [128, 1]), ("d1bc", [128, 4]),
        ("B3s", [128, 128]), ("B2d", [128, 128]),
        ("G1", [128, 128]), ("B1n", [128, 128]),
        ("G2", [128, 128]), ("B2n", [128, 128]),
        ("IOTA", [128, 128]), ("IDN", [128, 128]), ("EPSc", [128, 1]),
    ]:
        wd[name] = nc.dram_tensor(name, shape, f32, kind="ExternalInput").ap()
    out_d = nc.dram_tensor("out", [W_PER_CORE * 128, H], f32,
                           kind="ExternalOutput").ap()

    with tile.TileContext(nc) as tc, ExitStack() as ctx:
        const = ctx.enter_context(tc.tile_pool(name="const", bufs=1))
        ct = {}
        for name, ap in wd.items():
            ct[name] = const.tile(list(ap.shape), f32, tag=name,
                                  name=f"c_{name}")
            nc.sync.dma_start(ct[name][:], ap[:])

        hEp = ctx.enter_context(tc.tile_pool(name="hEp", bufs=4))
        sbp = ctx.enter_context(tc.tile_pool(name="sbp", bufs=3))
        msgp = ctx.enter_context(tc.tile_pool(name="msgp", bufs=3))
        ohp = ctx.enter_context(tc.tile_pool(name="ohp", bufs=4))
        srcp = ctx.enter_context(tc.tile_pool(name="srcp", bufs=2))
        nodep = ctx.enter_context(tc.tile_pool(name="nodep", bufs=2))
        colp = ctx.enter_context(tc.tile_pool(name="colp", bufs=4))
        pmA = ctx.enter_context(tc.tile_pool(name="pmA", bufs=2, space=PSUM))
        pmB = ctx.enter_context(tc.tile_pool(name="pmB", bufs=2, space=PSUM))
        pmM = ctx.enter_context(tc.tile_pool(name="pmM", bufs=2, space=PSUM))
        pmS = ctx.enter_context(tc.tile_pool(name="pmS", bufs=2, space=PSUM))

        def layer_norm(u, gt, bt, out_tag):
            ms = colp.tile([128, 1], f32, tag="ms")
            nc.vector.reduce_sum(ms[:], u[:], axis=AX)
            sq = nodep.tile([128, 128], f32, tag="sq")
            qs = colp.tile([128, 1], f32, tag="qs")
            nc.scalar.activation(sq[:], u[:], AF.Square, accum_out=qs[:])
            mc = colp.tile([128, 1], f32, tag="mc")
            nc.vector.tensor_scalar(mc[:], ms[:], 1.0 / H, None, OP.mult)
            msq = colp.tile([128, 1], f32, tag="msq")
            nc.vector.tensor_mul(msq[:], mc[:], mc[:])
            var = colp.tile([128, 1], f32, tag="var")
            nc.vector.tensor_scalar(var[:], qs[:], 1.0 / H, None, OP.mult)
            nc.vector.tensor_sub(var[:], var[:], msq[:])
            sd = colp.tile([128, 1], f32, tag="sd")
            nc.scalar.activation(sd[:], var[:], AF.Sqrt, bias=ct["EPSc"][:])
            rs = colp.tile([128, 1], f32, tag="rs")
            nc.vector.reciprocal(rs[:], sd[:])
            xn = nodep.tile([128, 128], f32, tag="xn")
            nc.vector.tensor_scalar(xn[:], u[:], mc[:], rs[:],
                                    OP.subtract, OP.mult)
            y = nodep.tile([128, 128], f32, tag=out_tag)
            nc.vector.tensor_mul(y[:], xn[:], gt[:])
            nc.vector.tensor_add(y[:], y[:], bt[:])
            return y

        for w in [w for _ in range(repeat) for w in range(W_PER_CORE)]:
            srcw = srcp.tile([128, T_win], f32)
            nc.sync.dma_start(srcw[:], src_d[w])
            degc = colp.tile([128, 1], f32, tag="deg")
            nc.sync.dma_start(degc[:], deg_d[w])
            pseg = pmS.tile([128, 128], f32, tag="s")

            # ---- edge phase: 512-edge blocks
            for s in range(SPB):
                b = w * SPB + s
                het = hEp.tile([128, 2, 512], f32, tag="he")
                nc.sync.dma_start(
                    het[:], hE_d[b].rearrange("(c p) e -> p c e", p=128))
                pm1 = pmA.tile([128, 512], f32, tag="a")
                nc.tensor.matmul(pm1[:], ct["W1s"][:, 0, :], het[:, 0, :],
                                 start=True, stop=False)
                nc.tensor.matmul(pm1[:], ct["W1s"][:, 1, :], het[:, 1, :],
                                 start=False, stop=True)
                g1 = sbp.tile([128, 512], f32, tag="g1")
                nc.scalar.activation(g1[:], pm1[:], AF.Gelu, bias=ct["b1c"][:])
                pm2 = pmB.tile([128, 512], f32, tag="b")
                nc.tensor.matmul(pm2[:], ct["W2s"][:], g1[:],
                                 start=True, stop=True)
                g2 = sbp.tile([128, 512], f32, tag="g2")
                nc.scalar.activation(g2[:], pm2[:], AF.Gelu, bias=ct["b2c"][:])
                pmsg = pmM.tile([128, 4, 128], f32, tag="m")
                for k in range(4):
                    nc.tensor.matmul(pmsg[:, k, :],
                                     g2[:, k * 128:(k + 1) * 128],
                                     ct["W3s"][:], start=True, stop=True)
                msg = msgp.tile([128, 4, 128], f32)
                nc.vector.tensor_copy(msg[:], pmsg[:])
                for k in range(4):
                    t = s * 4 + k
                    oh = ohp.tile([128, 128], f32)
                    nc.vector.tensor_scalar(oh[:], ct["IOTA"][:],
                                            srcw[:, t:t + 1], None,
                                            OP.is_equal)
                    nc.tensor.matmul(pseg[:], oh[:], msg[:, k, :],
                                     start=(s == 0 and k == 0),
                                     stop=(s == SPB - 1 and k == 3))

            # ---- node phase
            dh = nodep.tile([128, 128], f32, tag="dh")
            nc.vector.tensor_copy(dh[:], pseg[:])
            hv = nodep.tile([128, 128], f32, tag="hv")
            nc.sync.dma_start(hv[:], hV_d[w * 128:(w + 1) * 128, :])
            t0 = nodep.tile([128, 128], f32, tag="t0")
            nc.vector.tensor_scalar(t0[:], ct["B3s"][:], degc[:], None,
                                    OP.mult)
            u = nodep.tile([128, 128], f32, tag="u")
            nc.vector.tensor_add(u[:], hv[:], dh[:])
            nc.vector.tensor_add(u[:], u[:], t0[:])
            y = layer_norm(u, ct["G1"], ct["B1n"], "y")

            pyT = pmB.tile([128, 128], f32, tag="b")
            nc.tensor.transpose(pyT[:], y[:], ct["IDN"][:])
            yT = nodep.tile([128, 128], f32, tag="yT")
            nc.vector.tensor_copy(yT[:], pyT[:])
            pz1 = pmA.tile([128, 4, 128], f32, tag="a")
            for c in range(4):
                nc.tensor.matmul(pz1[:, c, :],
                                 ct["d1s"][:, c * 128:(c + 1) * 128], yT[:],
                                 start=True, stop=True)
            g1n = nodep.tile([128, 4, 128], f32, tag="g1n")
            for c in range(4):
                nc.scalar.activation(g1n[:, c, :], pz1[:, c, :], AF.Gelu,
                                     bias=ct["d1bc"][:, c:c + 1])
            pz2 = pmM.tile([128, 128], f32, tag="m")
            for c in range(4):
                nc.tensor.matmul(pz2[:], g1n[:, c, :], ct["d2s"][:, c, :],
                                 start=(c == 0), stop=(c == 3))
            x2 = nodep.tile([128, 128], f32, tag="x2")
            nc.vector.tensor_add(x2[:], y[:], pz2[:])
            nc.vector.tensor_add(x2[:], x2[:], ct["B2d"][:])
            yo = layer_norm(x2, ct["G2"], ct["B2n"], "yo")
            nc.sync.dma_start(out_d[w * 128:(w + 1) * 128, :], yo[:])

    nc.compile()
    _BUILD_CACHE[(T_win, repeat)] = nc
    return nc


# ------------------------------------------------------------------- driver

def run_device(p, wts, **spmd_kwargs):
    from concourse.bass_utils import run_bass_kernel_spmd

    nc = build_nc(p["T_win"])
    in_maps = []
    for c in range(NCORES):
        m = {"hE": p["hEb"][c], "srcf": p["srcf"][c],
             "hV": p["hVp"][c], "deg": p["degf"][c]}
        m.update(wts)
        in_maps.append(m)
    res = run_bass_kernel_spmd(nc, in_maps, list(range(NCORES)),
                               **spmd_kwargs)
    outs = np.stack([res.results[c]["out"] for c in range(NCORES)])
    outs = outs.reshape(NW, 128, H)
    out_full = np.empty((N_NODES, H), np.float32)
    out_full[p["perm"][p["pm"]]] = outs[p["pm"]]
    return out_full, res


def kernel(h_V, h_E, edge_idx, W1_w, W1_b, W2_w, W2_b, W3_w, W3_b,
           n1_g, n1_b, d1_w, d1_b, d2_w, d2_b, n2_g, n2_b):
    p = prep(h_V, h_E, edge_idx)
    wts = _weight_arrays(W1_w, W1_b, W2_w, W2_b, W3_w, W3_b,
                         n1_g, n1_b, d1_w, d1_b, d2_w, d2_b, n2_g, n2_b)
    out, _ = run_device(p, wts)
    return out
